# revision 1
# baseline (speedup 1.0000x reference)
"""GAT 3-layer kernel for 8 TRN2 NeuronCores (Bass/Tile).

Sharding: dst-node blocks of 6250 nodes/core (graph parallel per the hint).
Edges are routed to the core owning their dst node and sorted by dst.
Per layer:
  node phase: h = x@W and per-node attention scores s = x@(W@A) for the
    core's own nodes, staged as 512B table rows [h bf16*128 | s_src f32*4],
    AllGather -> full table in DRAM.
  edge phase: dma_gather rows by src (int16 indices -> lo/hi half-table
    split, two passes), per-128-edge chunk: one-hot dst matrix M via
    tensor_scalar(is_equal), segment-softmax WITHOUT max-subtraction
    (scores bounded), denominator folded as a 132nd matmul column:
       PSUM[d, 0:128] += M^T @ (ex (x) h_src);  PSUM[d, 128:132] += M^T @ ex
    s_dst per edge via telescoped range matmul: R_T[d,e] = (e >= start_d),
    s_dst_exp = R_T.T @ (K @ s_d) with K the first-difference matrix.
  postprocess: out = (1/4) sum_h NUM_h/(den_h+1e-16) + b; AllGather o.
"""

import math
import numpy as np

N = 50000
E = 800000
HEADS = 4
C = 32
NEG = 0.2
NCORES = 8
NB = 6250
BLK = 6272
NTAB = BLK * NCORES   # 50176
HALF = NTAB // 2      # 25088 = 4 blocks
NTILE = BLK // 128    # 49
PADROW = 6250         # junk row (half-table relative) for padding edges
BATCH_CH = 64         # chunks per dma_gather
SC_BATCH = 16         # chunks per score batch

_CACHE = {}


def _host_prep(edge_index):
    src = np.asarray(edge_index[0], dtype=np.int64)
    dst = np.asarray(edge_index[1], dtype=np.int64)
    loops = np.arange(N, dtype=np.int64)
    src = np.concatenate([src, loops])
    dst = np.concatenate([dst, loops])
    rowidx = (src // NB) * BLK + (src % NB)

    lists = [[[None, None] for _ in range(NTILE)] for _ in range(NCORES)]
    counts = np.zeros((NCORES, NTILE, 2), dtype=np.int64)
    for k in range(NCORES):
        m = (dst // NB) == k
        s_r = rowidx[m]
        d_l = dst[m] - k * NB
        order = np.argsort(d_l, kind="stable")
        s_r, d_l = s_r[order], d_l[order]
        t_of = d_l // 128
        for t in range(NTILE):
            mt = t_of == t
            sr_t, dl_t = s_r[mt], d_l[mt] - t * 128
            lo = sr_t < HALF
            for s in range(2):
                ms = lo if s == 0 else ~lo
                sr = sr_t[ms]
                lists[k][t][s] = (sr, dl_t[ms])
                counts[k, t, s] = sr.shape[0]

    nch_ts = np.maximum(1, np.ceil(counts.max(axis=0) / 128)).astype(np.int64)

    seq = []
    for s in range(2):
        for t in range(NTILE):
            for c in range(int(nch_ts[t, s])):
                seq.append((t, s, c))
    nchunk = len(seq)
    etot = nchunk * 128

    batches = []
    i = 0
    while i < nchunk:
        s = seq[i][1]
        j = i
        while j < nchunk and seq[j][1] == s and j - i < BATCH_CH:
            j += 1
        batches.append((i, j - i, s))
        i = j

    idx_w = np.zeros((NCORES, 128, nchunk), dtype=np.int32)
    dcol = np.zeros((NCORES, 128, nchunk), dtype=np.float32)
    estart = np.zeros((NCORES, 128, NTILE * 2), dtype=np.float32)
    for k in range(NCORES):
        flat_idx = np.full(etot, PADROW, dtype=np.int64)
        flat_dl = np.full(etot, 127, dtype=np.int64)
        pos = 0
        for s in range(2):
            for t in range(NTILE):
                sr, dl = lists[k][t][s]
                n = sr.shape[0]
                cap = int(nch_ts[t, s]) * 128
                flat_idx[pos:pos + n] = sr
                flat_dl[pos:pos + n] = dl
                st = np.searchsorted(dl, np.arange(128), side="left")
                estart[k, :, t * 2 + s] = st.astype(np.float32)
                pos += cap
        assert pos == etot
        idx_w[k] = flat_idx.reshape(nchunk, 128).T.astype(np.int32)
        dcol[k] = flat_dl.reshape(nchunk, 128).T.astype(np.float32)

    meta = dict(nch_ts=nch_ts, seq=seq, nchunk=nchunk, etot=etot,
                batches=batches)
    return idx_w, dcol, estart, meta


def _to_bf16(x):
    import ml_dtypes
    return np.asarray(x, dtype=np.float32).astype(ml_dtypes.bfloat16)


def _host_weights(inputs):
    outs = {}
    bt = np.zeros((128, 3 * C), dtype=np.float32)
    for l in range(1, 4):
        W = np.asarray(inputs[f"W{l}"], dtype=np.float32)
        a_s = np.asarray(inputs[f"a_src{l}"], dtype=np.float32)
        a_d = np.asarray(inputs[f"a_dst{l}"], dtype=np.float32)
        A = np.zeros((HEADS * C, 8), dtype=np.float32)
        for h in range(HEADS):
            A[h * C:(h + 1) * C, h] = a_s[h]
            A[h * C:(h + 1) * C, 4 + h] = a_d[h]
        WWA = np.concatenate([W, W @ A], axis=1)  # [din, 136]
        pad = np.zeros((128, 136), dtype=np.float32)
        pad[:W.shape[0]] = WWA
        outs[f"wwa{l}"] = _to_bf16(pad)
        bt[:, (l - 1) * C:l * C] = np.asarray(inputs[f"b{l}"], np.float32)[None, :]
    outs["btile"] = bt
    return outs


def _build_program(meta):
    import concourse.bass as bass
    import concourse.bacc as bacc
    import concourse.mybir as mybir
    import concourse.tile as tile
    from concourse import library_config

    fp32 = mybir.dt.float32
    bf16 = mybir.dt.bfloat16
    i32 = mybir.dt.int32
    AF = mybir.ActivationFunctionType
    OP = mybir.AluOpType

    nchunk = meta["nchunk"]
    etot = meta["etot"]
    nch_ts = meta["nch_ts"]
    seq = meta["seq"]
    batches = meta["batches"]
    emax = int(nch_ts.max()) * 128

    nc = bacc.Bacc("TRN2")
    xT = nc.declare_dram_parameter("xT", [128, BLK], bf16, isOutput=False)
    idxs_d = nc.declare_dram_parameter("idxs", [128, nchunk], i32, isOutput=False)
    dcol_d = nc.declare_dram_parameter("dcol", [128, nchunk], fp32, isOutput=False)
    est_d = nc.declare_dram_parameter("estart", [128, NTILE * 2], fp32, isOutput=False)
    wwa_d = [nc.declare_dram_parameter(f"wwa{l}", [128, 136], bf16, isOutput=False)
             for l in (1, 2, 3)]
    bt_d = nc.declare_dram_parameter("btile", [128, 3 * C], fp32, isOutput=False)
    iota_d = nc.declare_dram_parameter("iotas", [128, emax], fp32, isOutput=False)
    kt_d = nc.declare_dram_parameter("kt", [128, 128], bf16, isOutput=False)
    out_d = nc.declare_dram_parameter("out", [BLK, C], fp32, isOutput=True)

    tab_loc = nc.dram_tensor("tab_loc", [BLK, 256], bf16)
    tab_full = nc.dram_tensor("tab_full", [NTAB, 256], bf16, addr_space="Shared")

    with tile.TileContext(nc) as tc:
        with (
            tc.tile_pool(name="const", bufs=1) as cpool,
            tc.tile_pool(name="stage", bufs=3) as spool,
            tc.tile_pool(name="gbuf", bufs=2) as gpool,
            tc.tile_pool(name="work", bufs=3) as wpool,
            tc.tile_pool(name="rtp", bufs=2) as rtpool,
            tc.tile_pool(name="sc", bufs=3) as scpool,
            tc.tile_pool(name="acc", bufs=1) as apool,
            tc.tile_pool(name="psn", bufs=2, space="PSUM") as psn,
            tc.tile_pool(name="psd", bufs=1, space="PSUM") as psd,
            tc.tile_pool(name="pssc", bufs=2, space="PSUM") as pssc,
            tc.tile_pool(name="pstr", bufs=1, space="PSUM") as pstr,
            tc.tile_pool(name="psagg", bufs=2, space="PSUM") as aggpool,
        ):
            iotaB = cpool.tile([128, emax], fp32, tag="iotaB")
            nc.sync.dma_start(out=iotaB[:], in_=iota_d[:])
            ktile = cpool.tile([128, 128], bf16, tag="kt")
            nc.sync.dma_start(out=ktile[:], in_=kt_d[:])
            btile = cpool.tile([128, 3 * C], fp32, tag="btile")
            nc.sync.dma_start(out=btile[:], in_=bt_d[:])
            idxs = cpool.tile([128, nchunk], i32, tag="idxs")
            nc.sync.dma_start(out=idxs[:], in_=idxs_d[:])
            dcol = cpool.tile([128, nchunk], fp32, tag="dcol")
            nc.sync.dma_start(out=dcol[:], in_=dcol_d[:])
            estart = cpool.tile([128, NTILE * 2], fp32, tag="estart")
            nc.sync.dma_start(out=estart[:], in_=est_d[:])
            wwa = []
            for l in range(3):
                w = cpool.tile([128, 136], bf16, tag=f"wwa{l}")
                nc.sync.dma_start(out=w[:], in_=wwa_d[l][:])
                wwa.append(w)
            xTs = cpool.tile([128, BLK], bf16, tag="xTs")
            nc.sync.dma_start(out=xTs[:], in_=xT[:])
            negfix = cpool.tile([32, 4], fp32, tag="negfix")
            nc.vector.memset(negfix[:], -10000.0)

            from concourse.masks import make_identity
            ident = cpool.tile([128, 128], fp32, tag="ident")
            make_identity(nc, ident[:])
            sdst = apool.tile([128, NTILE * 4], bf16, tag="sd")       # delta-s
            accum = apool.tile([128, NTILE * 132], fp32, tag="accum")
            oT_sb = apool.tile([32, BLK], bf16, tag="oT")

            for layer in range(3):
                din = 128 if layer == 0 else C
                # ---------- node phase ----------
                for n in range(NTILE):
                    if layer == 0:
                        lhs_ap = xTs[:, n * 128:(n + 1) * 128]
                    else:
                        lhs_ap = oT_sb[:, n * 128:(n + 1) * 128]
                    ps = psn.tile([128, 136], fp32, tag="nodeps")
                    nc.tensor.matmul(out=ps[:], lhsT=lhs_ap[:din, :],
                                     rhs=wwa[layer][:din, :], start=True, stop=True)
                    tabst = spool.tile([128, 256], bf16, tag="tabst")
                    nc.scalar.copy(out=tabst[:, 0:128], in_=ps[:, 0:128])
                    nc.vector.tensor_copy(
                        out=tabst[:].bitcast(fp32)[:, 64:68], in_=ps[:, 128:132])
                    # delta-s for this tile: K @ s_d
                    psdt = psd.tile([128, 4], fp32, tag="dsps")
                    sdl = spool.tile([128, 4], bf16, tag="sdl")
                    nc.vector.tensor_copy(out=sdl[:], in_=ps[:, 132:136])
                    nc.tensor.matmul(out=psdt[:], lhsT=ktile[:], rhs=sdl[:],
                                     start=True, stop=True)
                    nc.vector.tensor_copy(out=sdst[:, n * 4:(n + 1) * 4], in_=psdt[:])
                    nc.sync.dma_start(out=tab_loc[n * 128:(n + 1) * 128, :],
                                      in_=tabst[:])
                # junk rows 6250..6271: kill pad-edge scores (s_s = -1e4)
                nc.sync.dma_start(
                    out=tab_loc.bitcast(fp32)[6250:6272, 64:68],
                    in_=negfix[:22, :])
                nc.gpsimd.collective_compute(
                    "AllGather", mybir.AluOpType.bypass,
                    replica_groups=[list(range(NCORES))],
                    ins=[tab_loc[:]], outs=[tab_full[:]])

                # ---------- edge phase ----------
                nc.vector.memset(accum[:], 0.0)
                chunk_batch = {}
                for (bstart, bnch, bs) in batches:
                    for c in range(bstart, bstart + bnch):
                        chunk_batch[c] = bstart
                g_of = {}

                def ensure_gather(c):
                    bstart = chunk_batch[c]
                    if bstart not in g_of:
                        for (bs2, bn2, _s2) in batches:
                            if bs2 == bstart:
                                bnch = bn2
                                break
                        g = gpool.tile([128, BATCH_CH, 256], bf16, tag="g")
                        for ci in range(bnch):
                            nc.gpsimd.indirect_dma_start(
                                out=g[:, ci, :], out_offset=None,
                                in_=tab_full[:],
                                in_offset=bass.IndirectOffsetOnAxis(
                                    ap=idxs[:, bstart + ci:bstart + ci + 1],
                                    axis=0))
                        g_of[bstart] = g
                    return g_of[bstart], bstart

                cidx0 = 0
                for s in range(2):
                    for t in range(NTILE):
                        nch = int(nch_ts[t, s])
                        ets = nch * 128
                        rt = rtpool.tile([128, emax], bf16, tag="rt")
                        nc.vector.tensor_tensor(
                            out=rt[:, :ets], in0=iotaB[:, :ets],
                            in1=estart[:, t * 2 + s:t * 2 + s + 1]
                            .to_broadcast([128, ets]), op=OP.is_ge)
                        pagg = aggpool.tile([128, 132], fp32, tag="agg")
                        for sb0 in range(0, nch, SC_BATCH):
                            sn = min(SC_BATCH, nch - sb0)
                            ps_sc = pssc.tile([128, 64], fp32, tag="scps")
                            for j in range(sn):
                                cl = sb0 + j
                                ensure_gather(cidx0 + cl)
                                nc.tensor.matmul(
                                    out=ps_sc[:, j * 4:(j + 1) * 4],
                                    lhsT=rt[:, cl * 128:(cl + 1) * 128],
                                    rhs=sdst[:, t * 4:(t + 1) * 4],
                                    start=True, stop=True,
                                    skip_group_check=True)
                            sc_sb = scpool.tile([128, 64], fp32, tag="scsb")
                            for j in range(sn):
                                c = cidx0 + sb0 + j
                                g, bstart = ensure_gather(c)
                                gf32 = g[:].bitcast(fp32)
                                nc.vector.tensor_tensor(
                                    out=sc_sb[:, j * 4:(j + 1) * 4],
                                    in0=ps_sc[:, j * 4:(j + 1) * 4],
                                    in1=gf32[:, c - bstart, 64:68], op=OP.add)
                            t1 = scpool.tile([128, 64], fp32, tag="t1")
                            nc.vector.tensor_scalar(
                                out=t1[:, :sn * 4], in0=sc_sb[:, :sn * 4],
                                scalar1=NEG, scalar2=None, op0=OP.mult)
                            nc.vector.tensor_tensor(
                                out=sc_sb[:, :sn * 4], in0=sc_sb[:, :sn * 4],
                                in1=t1[:, :sn * 4], op=OP.max)
                            ex_sb = scpool.tile([128, 64], bf16, tag="exsb")
                            nc.scalar.activation(out=ex_sb[:, :sn * 4],
                                                 in_=sc_sb[:, :sn * 4],
                                                 func=AF.Exp)
                            for j in range(sn):
                                cl = sb0 + j
                                c = cidx0 + cl
                                g, bstart = ensure_gather(c)
                                off = c - bstart
                                vx = wpool.tile([128, 132], bf16, tag="vx")
                                nc.vector.tensor_tensor(
                                    out=vx[:, 0:128].rearrange(
                                        "p (h c) -> p h c", h=4),
                                    in0=g[:, off, 0:128].rearrange(
                                        "p (h c) -> p h c", h=4),
                                    in1=ex_sb[:, j * 4:(j + 1) * 4, None]
                                    .to_broadcast([128, 4, 32]),
                                    op=OP.mult)
                                nc.vector.tensor_copy(
                                    out=vx[:, 128:132],
                                    in_=ex_sb[:, j * 4:(j + 1) * 4])
                                mt = wpool.tile([128, 128], bf16, tag="mt")
                                nc.vector.tensor_tensor(
                                    out=mt[:], in0=iotaB[:, :128],
                                    in1=dcol[:, c:c + 1].to_broadcast([128, 128]),
                                    op=OP.is_equal)
                                nc.tensor.matmul(
                                    out=pagg[:], lhsT=mt[:], rhs=vx[:],
                                    start=(cl == 0), stop=(cl == nch - 1),
                                    skip_group_check=True)
                                if cl == nch - 1:
                                    nc.vector.tensor_tensor(
                                        out=accum[:, t * 132:(t + 1) * 132],
                                        in0=accum[:, t * 132:(t + 1) * 132],
                                        in1=pagg[:], op=OP.add)
                        cidx0 += nch

                # ---------- postprocess ----------
                for t in range(NTILE):
                    num = accum[:, t * 132:t * 132 + 128]
                    den = accum[:, t * 132 + 128:t * 132 + 132]
                    rec = spool.tile([128, 4], fp32, tag="rec")
                    nc.vector.tensor_scalar(out=rec[:], in0=den[:], scalar1=1e-16,
                                            scalar2=None, op0=OP.add)
                    nc.vector.reciprocal(out=rec[:], in_=rec[:])
                    nc.vector.tensor_scalar(out=rec[:], in0=rec[:], scalar1=0.25,
                                            scalar2=None, op0=OP.mult)
                    scl = spool.tile([128, 128], fp32, tag="scl")
                    nc.vector.tensor_tensor(
                        out=scl[:].rearrange("p (h c) -> p h c", h=4),
                        in0=num.rearrange("p (h c) -> p h c", h=4),
                        in1=rec[:, :, None].to_broadcast([128, 4, 32]), op=OP.mult)
                    osum = spool.tile([128, C], fp32, tag="osum")
                    nc.vector.tensor_tensor(out=osum[:], in0=scl[:, 0:32],
                                            in1=scl[:, 32:64], op=OP.add)
                    nc.vector.tensor_tensor(out=osum[:], in0=osum[:],
                                            in1=scl[:, 64:96], op=OP.add)
                    nc.vector.tensor_tensor(out=osum[:], in0=osum[:],
                                            in1=scl[:, 96:128], op=OP.add)
                    nc.vector.tensor_tensor(
                        out=osum[:], in0=osum[:],
                        in1=btile[:, layer * C:(layer + 1) * C], op=OP.add)
                    if layer < 2:
                        pst = pstr.tile([32, 128], fp32, tag="pst")
                        nc.tensor.transpose(out=pst[:], in_=osum[:],
                                            identity=ident[:])
                        nc.vector.tensor_copy(
                            out=oT_sb[:, t * 128:(t + 1) * 128], in_=pst[:])
                    else:
                        nc.sync.dma_start(out=out_d[t * 128:(t + 1) * 128, :],
                                          in_=osum[:])
    nc.compile()
    return nc


def kernel(**inputs):
    import ml_dtypes
    from concourse.bass_utils import run_bass_kernel_spmd

    edge_index = np.asarray(inputs["edge_index"])
    if "prep" not in _CACHE:
        _CACHE["prep"] = _host_prep(edge_index)
        _CACHE["nc"] = _build_program(_CACHE["prep"][3])
    idx_w, dcol, estart, meta = _CACHE["prep"]
    nc = _CACHE["nc"]
    wdict = _host_weights(inputs)

    x = np.asarray(inputs["x"], dtype=np.float32)
    emax = int(meta["nch_ts"].max()) * 128
    iotas = np.broadcast_to(
        np.arange(emax, dtype=np.float32)[None, :], (128, emax)).copy()
    # K^T: K[d,d'] = delta(d'==d) - delta(d'==d-1) -> KT[d-1, d] = -1 (superdiag)
    kt = np.eye(128, dtype=np.float32)
    kt[np.arange(127), np.arange(1, 128)] = -1.0
    kt_b = kt.astype(ml_dtypes.bfloat16)

    in_maps = []
    for k in range(NCORES):
        xk = np.zeros((BLK, 128), dtype=np.float32)
        xk[:NB] = x[k * NB:(k + 1) * NB]
        m = {
            "xT": np.ascontiguousarray(xk.T).astype(ml_dtypes.bfloat16),
            "idxs": idx_w[k], "dcol": dcol[k], "estart": estart[k],
            "btile": wdict["btile"], "iotas": iotas, "kt": kt_b,
        }
        for l in (1, 2, 3):
            m[f"wwa{l}"] = wdict[f"wwa{l}"]
        in_maps.append(m)

    _CACHE["in_maps"] = in_maps
    res = run_bass_kernel_spmd(nc, in_maps, core_ids=list(range(NCORES)))
    out = np.zeros((N, C), dtype=np.float32)
    for k in range(NCORES):
        out[k * NB:(k + 1) * NB] = res.results[k]["out"][:NB]
    return out



# revision 2
# speedup vs baseline: 1.6247x; 1.6247x over previous
"""GAT 3-layer kernel for 8 TRN2 NeuronCores (Bass/Tile).

Sharding: dst-node blocks of 6250 nodes/core (graph parallel per the hint).
Edges are routed to the core owning their dst node and sorted by dst.

Per layer:
  node phase: h = x@W and per-node attention scores for the core's own
    nodes, staged as 512B table rows [h bf16*128 | s_src f32*4],
    AllGather -> full table in DRAM.
  edge phase: dma_gather rows by src, per-128-edge chunk: one-hot dst
    matrix M via is_equal, segment-softmax WITHOUT max-subtraction
    (scores bounded), denominator folded as four extra matmul columns:
       PSUM[d, 0:128] += M^T @ (ex (x) h_src);  PSUM[d, 128:132] += M^T @ ex
    s_dst per edge via telescoped range matmul: R_T[d,e] = (e >= start_d),
    s_dst = R_T.T @ (K @ s_d) with K the first-difference matrix.
  postprocess per dst tile: out = (1/4) sum_h NUM_h/(den_h+1e-16) + b.

Per-call I/O is minimized for the axon tunnel: the only runtime input is
the core's x shard (bf16, transposed) and the only output is the core's
out shard (bf16).  All edge-routing tables and weights are embedded in
the NEFF as compact inline constants ([8*128, .] stacks; each core picks
its 128-row slice with a partition_id-offset DMA) and expanded to
compute dtypes on device.  Iotas are generated on device.
"""

import hashlib
import numpy as np

N = 50000
E = 800000
HEADS = 4
C = 32
NEG = 0.2
NCORES = 8
NB = 6250
BLK = 6272
NTAB = BLK * NCORES   # 50176
NTILE = BLK // 128    # 49
PADROW = 6250         # absolute junk row (block 0 pad region)
WIN = 16              # chunks (of 128 edges) per gather/vector window

_CACHE = {}


def _host_prep(edge_index):
    src = np.asarray(edge_index[0], dtype=np.int64)
    dst = np.asarray(edge_index[1], dtype=np.int64)
    loops = np.arange(N, dtype=np.int64)
    src = np.concatenate([src, loops])
    dst = np.concatenate([dst, loops])
    rowidx = (src // NB) * BLK + (src % NB)   # absolute table row of src

    per_core = []
    counts = np.zeros((NCORES, NTILE), dtype=np.int64)
    for k in range(NCORES):
        m = (dst // NB) == k
        s_r = rowidx[m]
        d_l = dst[m] - k * NB
        order = np.argsort(d_l, kind="stable")
        s_r, d_l = s_r[order], d_l[order]
        t_of = d_l // 128
        tl = []
        for t in range(NTILE):
            mt = t_of == t
            tl.append((s_r[mt], d_l[mt] - t * 128))
            counts[k, t] = int(mt.sum())
        per_core.append(tl)

    nch_t = np.maximum(1, np.ceil(counts.max(axis=0) / 128)).astype(np.int64)
    nchunk = int(nch_t.sum())
    etot = nchunk * 128
    emax = int(nch_t.max()) * 128

    idx_w = np.zeros((NCORES, 128, nchunk), dtype=np.uint16)
    dcol = np.zeros((NCORES, 128, nchunk), dtype=np.uint8)
    estart = np.zeros((NCORES, 128, NTILE), dtype=np.uint16)
    for k in range(NCORES):
        flat_idx = np.full(etot, PADROW, dtype=np.int64)
        flat_dl = np.full(etot, 127, dtype=np.int64)
        pos = 0
        for t in range(NTILE):
            sr, dl = per_core[k][t]
            n = sr.shape[0]
            cap = int(nch_t[t]) * 128
            flat_idx[pos:pos + n] = sr
            flat_dl[pos:pos + n] = dl
            st = np.searchsorted(dl, np.arange(128), side="left")
            estart[k, :, t] = st.astype(np.uint16)
            pos += cap
        assert pos == etot
        idx_w[k] = flat_idx.reshape(nchunk, 128).T.astype(np.uint16)
        dcol[k] = flat_dl.reshape(nchunk, 128).T.astype(np.uint8)

    meta = dict(nch_t=nch_t, nchunk=nchunk, emax=emax)
    return idx_w, dcol, estart, meta


def _to_bf16(x):
    import ml_dtypes
    return np.asarray(x, dtype=np.float32).astype(ml_dtypes.bfloat16)


def _host_weights(inputs):
    outs = {}
    bt = np.zeros((128, 3 * C), dtype=np.float32)
    for l in range(1, 4):
        W = np.asarray(inputs[f"W{l}"], dtype=np.float32)
        a_s = np.asarray(inputs[f"a_src{l}"], dtype=np.float32)
        a_d = np.asarray(inputs[f"a_dst{l}"], dtype=np.float32)
        A = np.zeros((HEADS * C, 8), dtype=np.float32)
        for h in range(HEADS):
            A[h * C:(h + 1) * C, h] = a_s[h]
            A[h * C:(h + 1) * C, 4 + h] = a_d[h]
        WWA = np.concatenate([W, W @ A], axis=1)  # [din, 136]
        pad = np.zeros((128, 136), dtype=np.float32)
        pad[:W.shape[0]] = WWA
        outs[f"wwa{l}"] = _to_bf16(pad)
        bt[:, (l - 1) * C:l * C] = np.asarray(inputs[f"b{l}"], np.float32)[None, :]
    outs["btile"] = bt
    return outs


def _build_program(meta, idx_w, dcol_w, est_w, wdict):
    import ml_dtypes
    import concourse.bass as bass
    import concourse.bacc as bacc
    import concourse.mybir as mybir
    import concourse.tile as tile
    from concourse.bass import ds

    fp32 = mybir.dt.float32
    bf16 = mybir.dt.bfloat16
    i32 = mybir.dt.int32
    u16 = mybir.dt.uint16
    u8 = mybir.dt.uint8
    AF = mybir.ActivationFunctionType
    OP = mybir.AluOpType

    nchunk = meta["nchunk"]
    nch_t = meta["nch_t"]
    emax = meta["emax"]

    # K^T: K[d,d'] = delta(d'==d) - delta(d'==d-1) -> KT[d-1, d] = -1
    kt = np.eye(128, dtype=np.float32)
    kt[np.arange(127), np.arange(1, 128)] = -1.0

    nc = bacc.Bacc("TRN2")
    xT = nc.declare_dram_parameter("xT", [128, BLK], bf16, isOutput=False)
    out_d = nc.declare_dram_parameter("out", [BLK, C], bf16, isOutput=True)

    idx_all = nc.inline_tensor(
        np.ascontiguousarray(idx_w.reshape(NCORES * 128, nchunk)), name="idxa")
    dcol_all = nc.inline_tensor(
        np.ascontiguousarray(dcol_w.reshape(NCORES * 128, nchunk)), name="dcola")
    est_all = nc.inline_tensor(
        np.ascontiguousarray(est_w.reshape(NCORES * 128, NTILE)), name="esta")
    kt_d = nc.inline_tensor(kt.astype(ml_dtypes.bfloat16), name="ktc")
    bt_d = nc.inline_tensor(wdict["btile"], name="btc")
    wwa_d = [nc.inline_tensor(wdict[f"wwa{l}"], name=f"wwac{l}")
             for l in (1, 2, 3)]

    tab_loc = nc.dram_tensor("tab_loc", [BLK, 256], bf16)
    tab_full = nc.dram_tensor("tab_full", [NTAB, 256], bf16, addr_space="Shared")

    with tile.TileContext(nc) as tc:
        with (
            tc.tile_pool(name="const", bufs=1) as cpool,
            tc.tile_pool(name="stage", bufs=1) as stpool,
            tc.tile_pool(name="sp", bufs=3) as spool,
            tc.tile_pool(name="gbuf", bufs=3) as gpool,
            tc.tile_pool(name="work", bufs=3) as wpool,
            tc.tile_pool(name="rtp", bufs=2) as rtpool,
            tc.tile_pool(name="sc", bufs=3) as scpool,
            tc.tile_pool(name="psn", bufs=2, space="PSUM") as psn,
            tc.tile_pool(name="psd", bufs=1, space="PSUM") as psd,
            tc.tile_pool(name="pssc", bufs=2, space="PSUM") as pssc,
            tc.tile_pool(name="pstr", bufs=1, space="PSUM") as pstr,
            tc.tile_pool(name="psagg", bufs=2, space="PSUM") as aggpool,
        ):
            pid = nc.sync.partition_id()
            row0 = pid * 128

            # per-core tables: inline stacks -> own 128-row slice -> widen
            st_idx = stpool.tile([128, nchunk], u16, tag="st_idx")
            nc.sync.dma_start(out=st_idx[:], in_=idx_all[ds(row0, 128), :])
            idxs = cpool.tile([128, nchunk], i32, tag="idxs")
            nc.vector.tensor_copy(out=idxs[:], in_=st_idx[:])

            st_dc = stpool.tile([128, nchunk], u8, tag="st_dc")
            nc.sync.dma_start(out=st_dc[:], in_=dcol_all[ds(row0, 128), :])
            dcolf = cpool.tile([128, nchunk], fp32, tag="dcolf")
            nc.vector.tensor_copy(out=dcolf[:], in_=st_dc[:])

            st_es = stpool.tile([128, NTILE], u16, tag="st_es")
            nc.sync.dma_start(out=st_es[:], in_=est_all[ds(row0, 128), :])
            estf = cpool.tile([128, NTILE], fp32, tag="estf")
            nc.vector.tensor_copy(out=estf[:], in_=st_es[:])

            # iotas generated on device (verified-exact int path, then widen)
            st_ib = stpool.tile([128, emax], i32, tag="st_ib")
            nc.gpsimd.iota(out=st_ib[:], pattern=[[1, emax]], base=0,
                           channel_multiplier=0)
            iotaBf = cpool.tile([128, emax], fp32, tag="iotaBf")
            nc.vector.tensor_copy(out=iotaBf[:], in_=st_ib[:])

            st_ir = stpool.tile([128, WIN * 128], i32, tag="st_ir")
            nc.gpsimd.iota(out=st_ir[:], pattern=[[0, WIN], [1, 128]], base=0,
                           channel_multiplier=0)
            iotaRf = cpool.tile([128, WIN * 128], fp32, tag="iotaRf")
            nc.vector.tensor_copy(out=iotaRf[:], in_=st_ir[:])

            ktile = cpool.tile([128, 128], bf16, tag="kt")
            nc.sync.dma_start(out=ktile[:], in_=kt_d[:])
            btile = cpool.tile([128, 3 * C], fp32, tag="btile")
            nc.sync.dma_start(out=btile[:], in_=bt_d[:])
            wwa = []
            for l in range(3):
                w = cpool.tile([128, 136], bf16, tag=f"wwa{l}")
                nc.sync.dma_start(out=w[:], in_=wwa_d[l][:])
                wwa.append(w)
            xTs = cpool.tile([128, BLK], bf16, tag="xTs")
            nc.sync.dma_start(out=xTs[:], in_=xT[:])
            negfix = cpool.tile([32, 4], fp32, tag="negfix")
            nc.vector.memset(negfix[:], -10000.0)

            from concourse.masks import make_identity
            ident = cpool.tile([128, 128], fp32, tag="ident")
            make_identity(nc, ident[:])
            sdst = cpool.tile([128, NTILE * 4], bf16, tag="sd")   # delta-s
            oT_sb = cpool.tile([32, BLK], bf16, tag="oT")

            for layer in range(3):
                din = 128 if layer == 0 else C
                # ---------- node phase ----------
                for n in range(NTILE):
                    if layer == 0:
                        lhs_ap = xTs[:, n * 128:(n + 1) * 128]
                    else:
                        lhs_ap = oT_sb[:, n * 128:(n + 1) * 128]
                    ps = psn.tile([128, 136], fp32, tag="nodeps")
                    nc.tensor.matmul(out=ps[:], lhsT=lhs_ap[:din, :],
                                     rhs=wwa[layer][:din, :], start=True, stop=True)
                    tabst = spool.tile([128, 256], bf16, tag="tabst")
                    nc.scalar.copy(out=tabst[:, 0:128], in_=ps[:, 0:128])
                    nc.vector.tensor_copy(
                        out=tabst[:].bitcast(fp32)[:, 64:68], in_=ps[:, 128:132])
                    # delta-s for this tile: K @ s_d
                    psdt = psd.tile([128, 4], fp32, tag="dsps")
                    sdl = spool.tile([128, 4], bf16, tag="sdl")
                    nc.vector.tensor_copy(out=sdl[:], in_=ps[:, 132:136])
                    nc.tensor.matmul(out=psdt[:], lhsT=ktile[:], rhs=sdl[:],
                                     start=True, stop=True)
                    nc.vector.tensor_copy(out=sdst[:, n * 4:(n + 1) * 4], in_=psdt[:])
                    nc.sync.dma_start(out=tab_loc[n * 128:(n + 1) * 128, :],
                                      in_=tabst[:])
                # junk rows 6250..6271: kill pad-edge scores (s_s = -1e4)
                nc.sync.dma_start(
                    out=tab_loc.bitcast(fp32)[6250:6272, 64:68],
                    in_=negfix[:22, :])
                nc.gpsimd.collective_compute(
                    "AllGather", mybir.AluOpType.bypass,
                    replica_groups=[list(range(NCORES))],
                    ins=[tab_loc[:]], outs=[tab_full[:]])

                # ---------- edge phase ----------
                cbase = 0
                for t in range(NTILE):
                    nch = int(nch_t[t])
                    ets = nch * 128
                    rt = rtpool.tile([128, emax], bf16, tag="rt")
                    nc.vector.tensor_tensor(
                        out=rt[:, :ets], in0=iotaBf[:, :ets],
                        in1=estf[:, t:t + 1].to_broadcast([128, ets]), op=OP.is_ge)
                    pagg = aggpool.tile([128, 132], fp32, tag="agg")
                    for w0 in range(0, nch, WIN):
                        sn = min(WIN, nch - w0)
                        c0 = cbase + w0
                        g = gpool.tile([128, WIN, 256], bf16, tag="g")
                        for j in range(sn):
                            nc.gpsimd.indirect_dma_start(
                                out=g[:, j, :], out_offset=None,
                                in_=tab_full[:],
                                in_offset=bass.IndirectOffsetOnAxis(
                                    ap=idxs[:, c0 + j:c0 + j + 1], axis=0))
                        ps_sc = pssc.tile([128, WIN * 4], fp32, tag="scps")
                        for j in range(sn):
                            nc.tensor.matmul(
                                out=ps_sc[:, j * 4:(j + 1) * 4],
                                lhsT=rt[:, (w0 + j) * 128:(w0 + j + 1) * 128],
                                rhs=sdst[:, t * 4:(t + 1) * 4],
                                start=True, stop=True,
                                skip_group_check=True)
                        gf = g[:].bitcast(fp32)
                        sc_sb = scpool.tile([128, WIN * 4], fp32, tag="scsb")
                        nc.vector.tensor_tensor(
                            out=sc_sb[:, :sn * 4].rearrange("p (j h) -> p j h", j=sn),
                            in0=ps_sc[:, :sn * 4].rearrange("p (j h) -> p j h", j=sn),
                            in1=gf[:, 0:sn, 64:68], op=OP.add)
                        t1 = scpool.tile([128, WIN * 4], fp32, tag="t1")
                        nc.vector.tensor_scalar(
                            out=t1[:, :sn * 4], in0=sc_sb[:, :sn * 4],
                            scalar1=NEG, scalar2=None, op0=OP.mult)
                        nc.vector.tensor_tensor(
                            out=sc_sb[:, :sn * 4], in0=sc_sb[:, :sn * 4],
                            in1=t1[:, :sn * 4], op=OP.max)
                        ex_sb = scpool.tile([128, WIN * 4], bf16, tag="exsb")
                        nc.scalar.activation(out=ex_sb[:, :sn * 4],
                                             in_=sc_sb[:, :sn * 4],
                                             func=AF.Exp)
                        vx = wpool.tile([128, WIN, 132], bf16, tag="vx")
                        nc.vector.tensor_tensor(
                            out=vx[:, 0:sn, 0:128].rearrange(
                                "p j (h c) -> p j h c", h=4),
                            in0=g[:, 0:sn, 0:128].rearrange(
                                "p j (h c) -> p j h c", h=4),
                            in1=ex_sb[:, :sn * 4].rearrange(
                                "p (j h) -> p j h", j=sn)[:, :, :, None]
                            .to_broadcast([128, sn, 4, 32]),
                            op=OP.mult)
                        nc.vector.tensor_copy(
                            out=vx[:, 0:sn, 128:132],
                            in_=ex_sb[:, :sn * 4].rearrange("p (j h) -> p j h", j=sn))
                        mt = wpool.tile([128, WIN * 128], bf16, tag="mt")
                        nc.vector.tensor_tensor(
                            out=mt[:, :sn * 128].rearrange("p (j d) -> p j d", j=sn),
                            in0=iotaRf[:, :sn * 128].rearrange("p (j d) -> p j d", j=sn),
                            in1=dcolf[:, c0:c0 + sn, None].to_broadcast([128, sn, 128]),
                            op=OP.is_equal)
                        for j in range(sn):
                            cl = w0 + j
                            nc.tensor.matmul(
                                out=pagg[:], lhsT=mt[:, j * 128:(j + 1) * 128],
                                rhs=vx[:, j, :],
                                start=(cl == 0), stop=(cl == nch - 1),
                                skip_group_check=True)
                    cbase += nch

                    # ---------- postprocess tile t ----------
                    num = pagg[:, 0:128]
                    den = pagg[:, 128:132]
                    rec = spool.tile([128, 4], fp32, tag="rec")
                    nc.vector.tensor_scalar(out=rec[:], in0=den[:], scalar1=1e-16,
                                            scalar2=None, op0=OP.add)
                    nc.vector.reciprocal(out=rec[:], in_=rec[:])
                    nc.vector.tensor_scalar(out=rec[:], in0=rec[:], scalar1=0.25,
                                            scalar2=None, op0=OP.mult)
                    scl = spool.tile([128, 128], fp32, tag="scl")
                    nc.vector.tensor_tensor(
                        out=scl[:].rearrange("p (h c) -> p h c", h=4),
                        in0=num.rearrange("p (h c) -> p h c", h=4),
                        in1=rec[:, :, None].to_broadcast([128, 4, 32]), op=OP.mult)
                    osum = spool.tile([128, C], fp32, tag="osum")
                    nc.vector.tensor_tensor(out=osum[:], in0=scl[:, 0:32],
                                            in1=scl[:, 32:64], op=OP.add)
                    nc.vector.tensor_tensor(out=osum[:], in0=osum[:],
                                            in1=scl[:, 64:96], op=OP.add)
                    nc.vector.tensor_tensor(out=osum[:], in0=osum[:],
                                            in1=scl[:, 96:128], op=OP.add)
                    nc.vector.tensor_tensor(
                        out=osum[:], in0=osum[:],
                        in1=btile[:, layer * C:(layer + 1) * C], op=OP.add)
                    if layer < 2:
                        pst = pstr.tile([32, 128], fp32, tag="pst")
                        nc.tensor.transpose(out=pst[:], in_=osum[:],
                                            identity=ident[:])
                        nc.vector.tensor_copy(
                            out=oT_sb[:, t * 128:(t + 1) * 128], in_=pst[:])
                    else:
                        obf = spool.tile([128, C], bf16, tag="obf")
                        nc.vector.tensor_copy(out=obf[:], in_=osum[:])
                        nc.sync.dma_start(out=out_d[t * 128:(t + 1) * 128, :],
                                          in_=obf[:])
    nc.compile()
    return nc


def _inputs_key(inputs):
    h = hashlib.sha1()
    for name in ("edge_index", "W1", "a_src1", "a_dst1", "b1", "W2", "a_src2",
                 "a_dst2", "b2", "W3", "a_src3", "a_dst3", "b3"):
        h.update(np.ascontiguousarray(np.asarray(inputs[name])).tobytes())
    return h.hexdigest()


def kernel(**inputs):
    import ml_dtypes
    from concourse.bass_utils import run_bass_kernel_spmd

    key = _inputs_key(inputs)
    if _CACHE.get("key") != key:
        idx_w, dcol_w, est_w, meta = _host_prep(np.asarray(inputs["edge_index"]))
        wdict = _host_weights(inputs)
        nc = _build_program(meta, idx_w, dcol_w, est_w, wdict)
        _CACHE.update(key=key, nc=nc)
    nc = _CACHE["nc"]

    x = np.asarray(inputs["x"], dtype=np.float32)
    in_maps = []
    for k in range(NCORES):
        xk = np.zeros((BLK, 128), dtype=np.float32)
        xk[:NB] = x[k * NB:(k + 1) * NB]
        in_maps.append(
            {"xT": np.ascontiguousarray(xk.T).astype(ml_dtypes.bfloat16)})

    _CACHE["in_maps"] = in_maps
    res = run_bass_kernel_spmd(nc, in_maps, core_ids=list(range(NCORES)))
    out = np.zeros((N, C), dtype=np.float32)
    for k in range(NCORES):
        out[k * NB:(k + 1) * NB] = res.results[k]["out"][:NB].astype(np.float32)
    return out


# revision 3
# speedup vs baseline: 2.8624x; 1.7618x over previous
"""GAT 3-layer kernel for 8 TRN2 NeuronCores (Bass/Tile) — v3.

Same math as v2 (dst-block sharding, table AllGather, one-hot matmul
segment softmax) but the program is compressed with hardware For_i
loops: per-call jit/lower/compile/load overhead scales with BIR/NEFF
size, so ~300 instructions instead of ~13K is the big win.

Every dst tile's edge group is padded to a uniform NCH chunks of 128
edges, so the edge phase is one For_i over the 49 dst tiles per layer:
an idx-column stage copy, NCH indirect row gathers (static offset APs
into a fixed staging tile; dynamic content), NCH score matmuls, batched
vector softmax, NCH accumulating one-hot matmuls, inline postprocess.
Matmul lhsT requires static offsets, so dynamic slices are staged
through fixed tiles with DVE copies.

Per-call I/O: xT shard in (bf16), out shard out (bf16). Tables/weights
ride in the NEFF as compact inline constants; iotas are generated on
device.
"""

import hashlib
import numpy as np

N = 50000
E = 800000
HEADS = 4
C = 32
NEG = 0.2
NCORES = 8
NB = 6250
BLK = 6272
NTAB = BLK * NCORES   # 50176
NTILE = BLK // 128    # 49
PADROW = 6250         # absolute junk row (block 0 pad region)

_CACHE = {}


def _host_prep(edge_index):
    src = np.asarray(edge_index[0], dtype=np.int64)
    dst = np.asarray(edge_index[1], dtype=np.int64)
    loops = np.arange(N, dtype=np.int64)
    src = np.concatenate([src, loops])
    dst = np.concatenate([dst, loops])
    rowidx = (src // NB) * BLK + (src % NB)   # absolute table row of src

    per_core = []
    counts = np.zeros((NCORES, NTILE), dtype=np.int64)
    for k in range(NCORES):
        m = (dst // NB) == k
        s_r = rowidx[m]
        d_l = dst[m] - k * NB
        order = np.argsort(d_l, kind="stable")
        s_r, d_l = s_r[order], d_l[order]
        t_of = d_l // 128
        tl = []
        for t in range(NTILE):
            mt = t_of == t
            tl.append((s_r[mt], d_l[mt] - t * 128))
            counts[k, t] = int(mt.sum())
        per_core.append(tl)

    NCH = int(np.maximum(1, np.ceil(counts.max() / 128)))
    nchunk = NTILE * NCH
    nidx = NCH * 128

    idx_all = np.zeros((NCORES * 128, nchunk), dtype=np.uint16)
    dcol_all = np.zeros((NCORES * 128, nchunk), dtype=np.uint8)
    est_all = np.zeros((NCORES * 128, NTILE), dtype=np.uint16)
    for k in range(NCORES):
        for t in range(NTILE):
            sr, dl = per_core[k][t]
            n = sr.shape[0]
            flat_i = np.full(nidx, PADROW, dtype=np.int64)
            flat_d = np.full(nidx, 127, dtype=np.int64)
            flat_i[:n] = sr
            flat_d[:n] = dl
            idx_all[k * 128:(k + 1) * 128, t * NCH:(t + 1) * NCH] = \
                flat_i.reshape(NCH, 128).T.astype(np.uint16)
            dcol_all[k * 128:(k + 1) * 128, t * NCH:(t + 1) * NCH] = \
                flat_d.reshape(NCH, 128).T.astype(np.uint8)
            est_all[k * 128:(k + 1) * 128, t] = \
                np.searchsorted(dl, np.arange(128), side="left").astype(np.uint16)

    meta = dict(NCH=NCH)
    return idx_all, dcol_all, est_all, meta


def _to_bf16(x):
    import ml_dtypes
    return np.asarray(x, dtype=np.float32).astype(ml_dtypes.bfloat16)


def _host_weights(inputs):
    outs = {}
    bt = np.zeros((128, 3 * C), dtype=np.float32)
    for l in range(1, 4):
        W = np.asarray(inputs[f"W{l}"], dtype=np.float32)
        a_s = np.asarray(inputs[f"a_src{l}"], dtype=np.float32)
        a_d = np.asarray(inputs[f"a_dst{l}"], dtype=np.float32)
        A = np.zeros((HEADS * C, 8), dtype=np.float32)
        for h in range(HEADS):
            A[h * C:(h + 1) * C, h] = a_s[h]
            A[h * C:(h + 1) * C, 4 + h] = a_d[h]
        WWA = np.concatenate([W, W @ A], axis=1)  # [din, 136]
        pad = np.zeros((128, 136), dtype=np.float32)
        pad[:W.shape[0]] = WWA
        outs[f"wwa{l}"] = _to_bf16(pad)
        bt[:, (l - 1) * C:l * C] = np.asarray(inputs[f"b{l}"], np.float32)[None, :]
    outs["btile"] = bt
    return outs


def _build_program(meta, idx_all_np, dcol_all_np, est_all_np, wdict):
    import ml_dtypes
    import concourse.bass as bass
    import concourse.bacc as bacc
    import concourse.mybir as mybir
    import concourse.tile as tile
    from concourse.bass import ds

    fp32 = mybir.dt.float32
    bf16 = mybir.dt.bfloat16
    i32 = mybir.dt.int32
    u16 = mybir.dt.uint16
    u8 = mybir.dt.uint8
    AF = mybir.ActivationFunctionType
    OP = mybir.AluOpType

    NCH = meta["NCH"]
    nchunk = NTILE * NCH
    EW = NCH * 128            # edge-window width

    kt = np.eye(128, dtype=np.float32)
    kt[np.arange(127), np.arange(1, 128)] = -1.0

    nc = bacc.Bacc("TRN2")
    xT = nc.declare_dram_parameter("xT", [128, BLK], bf16, isOutput=False)
    out_d = nc.declare_dram_parameter("out", [BLK, C], bf16, isOutput=True)

    idx_d = nc.inline_tensor(idx_all_np, name="idxa")
    dcol_d = nc.inline_tensor(dcol_all_np, name="dcola")
    est_d = nc.inline_tensor(est_all_np, name="esta")
    kt_d = nc.inline_tensor(kt.astype(ml_dtypes.bfloat16), name="ktc")
    bt_d = nc.inline_tensor(wdict["btile"], name="btc")
    wwa_d = [nc.inline_tensor(wdict[f"wwa{l}"], name=f"wwac{l}")
             for l in (1, 2, 3)]

    tab_loc = nc.dram_tensor("tab_loc", [BLK, 256], bf16)
    tab_full = nc.dram_tensor("tab_full", [NTAB, 256], bf16, addr_space="Shared")

    with tile.TileContext(nc) as tc:
        with (
            tc.tile_pool(name="const", bufs=1) as cpool,
            tc.tile_pool(name="stage", bufs=1) as stpool,
            tc.tile_pool(name="sp", bufs=2) as spool,
            tc.tile_pool(name="nx", bufs=2) as nxpool,
            tc.tile_pool(name="gbuf", bufs=1) as gpool,
            tc.tile_pool(name="work", bufs=1) as wpool,
            tc.tile_pool(name="sc", bufs=1) as scpool,
            tc.tile_pool(name="psn", bufs=1, space="PSUM") as psn,
            tc.tile_pool(name="psd", bufs=1, space="PSUM") as psd,
            tc.tile_pool(name="pssc", bufs=1, space="PSUM") as pssc,
            tc.tile_pool(name="pstr", bufs=1, space="PSUM") as pstr,
            tc.tile_pool(name="psagg", bufs=1, space="PSUM") as aggpool,
        ):
            pid = nc.sync.partition_id()

            # --- per-core tables from inline stacks ---
            st_ix = stpool.tile([128, nchunk], u16, tag="st_ix")
            nc.sync.dma_start(out=st_ix[:], in_=idx_d[ds(pid * 128, 128), :])
            idxs = cpool.tile([128, nchunk], i32, tag="idxs")
            nc.vector.tensor_copy(out=idxs[:], in_=st_ix[:])

            st_dc = stpool.tile([128, nchunk], u8, tag="st_dc")
            nc.sync.dma_start(out=st_dc[:], in_=dcol_d[ds(pid * 128, 128), :])
            dcolf = cpool.tile([128, nchunk], fp32, tag="dcolf")
            nc.vector.tensor_copy(out=dcolf[:], in_=st_dc[:])

            st_es = stpool.tile([128, NTILE], u16, tag="st_es")
            nc.sync.dma_start(out=st_es[:], in_=est_d[ds(pid * 128, 128), :])
            estf = cpool.tile([128, NTILE], fp32, tag="estf")
            nc.vector.tensor_copy(out=estf[:], in_=st_es[:])

            # --- iotas (int path, exact) ---
            st_ib = stpool.tile([128, EW], i32, tag="st_ib")
            nc.gpsimd.iota(out=st_ib[:], pattern=[[1, EW]], base=0,
                           channel_multiplier=0)
            iotaBf = cpool.tile([128, EW], fp32, tag="iotaBf")
            nc.vector.tensor_copy(out=iotaBf[:], in_=st_ib[:])

            st_ir = stpool.tile([128, EW], i32, tag="st_ir")
            nc.gpsimd.iota(out=st_ir[:], pattern=[[0, NCH], [1, 128]], base=0,
                           channel_multiplier=0)
            iotaRf = cpool.tile([128, EW], fp32, tag="iotaRf")
            nc.vector.tensor_copy(out=iotaRf[:], in_=st_ir[:])

            ktile = cpool.tile([128, 128], bf16, tag="kt")
            nc.sync.dma_start(out=ktile[:], in_=kt_d[:])
            btile = cpool.tile([128, 3 * C], fp32, tag="btile")
            nc.sync.dma_start(out=btile[:], in_=bt_d[:])
            wwa = []
            for l in range(3):
                w = cpool.tile([128, 136], bf16, tag=f"wwa{l}")
                nc.sync.dma_start(out=w[:], in_=wwa_d[l][:])
                wwa.append(w)
            xTs = cpool.tile([128, BLK], bf16, tag="xTs")
            nc.sync.dma_start(out=xTs[:], in_=xT[:])
            negfix = cpool.tile([32, 4], fp32, tag="negfix")
            nc.vector.memset(negfix[:], -10000.0)

            from concourse.masks import make_identity
            ident = cpool.tile([128, 128], fp32, tag="ident")
            make_identity(nc, ident[:])
            sdst = cpool.tile([128, NTILE * 4], bf16, tag="sd")
            oT_sb = cpool.tile([32, BLK], bf16, tag="oT")

            for layer in range(3):
                din = 128 if layer == 0 else C
                # ---------- node phase ----------
                with tc.For_i(0, NTILE, name=f"node{layer}") as n:
                    xs = nxpool.tile([din, 128], bf16, tag=f"xs{layer}")
                    if layer == 0:
                        nc.vector.tensor_copy(out=xs[:],
                                              in_=xTs[:, ds(n * 128, 128)])
                    else:
                        nc.vector.tensor_copy(out=xs[:],
                                              in_=oT_sb[:, ds(n * 128, 128)])
                    ps = psn.tile([128, 136], fp32, tag="nodeps")
                    nc.tensor.matmul(out=ps[:], lhsT=xs[:],
                                     rhs=wwa[layer][:din, :], start=True, stop=True)
                    tabst = spool.tile([128, 256], bf16, tag="tabst")
                    nc.vector.memset(tabst[:, 136:256], 0.0)
                    nc.scalar.copy(out=tabst[:, 0:128], in_=ps[:, 0:128])
                    nc.vector.tensor_copy(
                        out=tabst[:].bitcast(fp32)[:, 64:68], in_=ps[:, 128:132])
                    psdt = psd.tile([128, 4], fp32, tag="dsps")
                    sdl = spool.tile([128, 4], bf16, tag="sdl")
                    nc.vector.tensor_copy(out=sdl[:], in_=ps[:, 132:136])
                    nc.tensor.matmul(out=psdt[:], lhsT=ktile[:], rhs=sdl[:],
                                     start=True, stop=True)
                    nc.vector.tensor_copy(out=sdst[:, ds(n * 4, 4)], in_=psdt[:])
                    nc.sync.dma_start(out=tab_loc[ds(n * 128, 128), :],
                                      in_=tabst[:])
                # junk rows 6250..6271: kill pad-edge scores (s_s = -1e4)
                nc.sync.dma_start(
                    out=tab_loc.bitcast(fp32)[6250:6272, 64:68],
                    in_=negfix[:22, :])
                nc.gpsimd.collective_compute(
                    "AllGather", mybir.AluOpType.bypass,
                    replica_groups=[list(range(NCORES))],
                    ins=[tab_loc[:]], outs=[tab_full[:]])

                # ---------- edge phase ----------
                with tc.For_i(0, NTILE, name=f"edge{layer}") as t:
                    sd_t = spool.tile([128, 4], bf16, tag="sdt")
                    nc.vector.tensor_copy(out=sd_t[:], in_=sdst[:, ds(t * 4, 4)])
                    idst = wpool.tile([128, NCH], i32, tag="idst")
                    nc.vector.tensor_copy(out=idst[:],
                                          in_=idxs[:, ds(t * NCH, NCH)])
                    g = gpool.tile([128, NCH, 256], bf16, tag="g")
                    for j in range(NCH):
                        nc.gpsimd.indirect_dma_start(
                            out=g[:, j, :], out_offset=None,
                            in_=tab_full[:],
                            in_offset=bass.IndirectOffsetOnAxis(
                                ap=idst[:, j:j + 1], axis=0))
                    rt = wpool.tile([128, EW], bf16, tag="rt")
                    nc.vector.tensor_tensor(
                        out=rt[:], in0=iotaBf[:],
                        in1=estf[:, ds(t, 1)].to_broadcast([128, EW]),
                        op=OP.is_ge)
                    ps_sc = pssc.tile([128, NCH * 4], fp32, tag="scps")
                    for j in range(NCH):
                        nc.tensor.matmul(
                            out=ps_sc[:, j * 4:(j + 1) * 4],
                            lhsT=rt[:, j * 128:(j + 1) * 128],
                            rhs=sd_t[:], start=True, stop=True,
                            skip_group_check=True)
                    gf = g[:].bitcast(fp32)
                    sc_sb = scpool.tile([128, NCH * 4], fp32, tag="scsb")
                    nc.vector.tensor_tensor(
                        out=sc_sb[:].rearrange("p (j h) -> p j h", j=NCH),
                        in0=ps_sc[:].rearrange("p (j h) -> p j h", j=NCH),
                        in1=gf[:, 0:NCH, 64:68], op=OP.add)
                    t1 = scpool.tile([128, NCH * 4], fp32, tag="t1")
                    nc.vector.tensor_scalar(
                        out=t1[:], in0=sc_sb[:],
                        scalar1=NEG, scalar2=None, op0=OP.mult)
                    nc.vector.tensor_tensor(
                        out=sc_sb[:], in0=sc_sb[:], in1=t1[:], op=OP.max)
                    ex_sb = scpool.tile([128, NCH * 4], bf16, tag="ex")
                    nc.scalar.activation(out=ex_sb[:], in_=sc_sb[:], func=AF.Exp)
                    vx = wpool.tile([128, NCH, 132], bf16, tag="vx")
                    nc.vector.tensor_tensor(
                        out=vx[:, :, 0:128].rearrange("p j (h c) -> p j h c", h=4),
                        in0=g[:, :, 0:128].rearrange("p j (h c) -> p j h c", h=4),
                        in1=ex_sb[:].rearrange(
                            "p (j h) -> p j h", j=NCH)[:, :, :, None]
                        .to_broadcast([128, NCH, 4, 32]),
                        op=OP.mult)
                    nc.vector.tensor_copy(
                        out=vx[:, :, 128:132],
                        in_=ex_sb[:].rearrange("p (j h) -> p j h", j=NCH))
                    mt = wpool.tile([128, EW], bf16, tag="mt")
                    nc.vector.tensor_tensor(
                        out=mt[:].rearrange("p (j d) -> p j d", j=NCH),
                        in0=iotaRf[:].rearrange("p (j d) -> p j d", j=NCH),
                        in1=dcolf[:, ds(t * NCH, NCH), None]
                        .to_broadcast([128, NCH, 128]),
                        op=OP.is_equal)
                    pagg = aggpool.tile([128, 132], fp32, tag="agg")
                    for j in range(NCH):
                        nc.tensor.matmul(
                            out=pagg[:], lhsT=mt[:, j * 128:(j + 1) * 128],
                            rhs=vx[:, j, :],
                            start=(j == 0), stop=(j == NCH - 1),
                            skip_group_check=True)

                    # ---------- postprocess tile t ----------
                    num = pagg[:, 0:128]
                    den = pagg[:, 128:132]
                    rec = spool.tile([128, 4], fp32, tag="rec")
                    nc.vector.tensor_scalar(out=rec[:], in0=den[:], scalar1=1e-16,
                                            scalar2=None, op0=OP.add)
                    nc.vector.reciprocal(out=rec[:], in_=rec[:])
                    nc.vector.tensor_scalar(out=rec[:], in0=rec[:], scalar1=0.25,
                                            scalar2=None, op0=OP.mult)
                    scl = spool.tile([128, 128], fp32, tag="scl")
                    nc.vector.tensor_tensor(
                        out=scl[:].rearrange("p (h c) -> p h c", h=4),
                        in0=num.rearrange("p (h c) -> p h c", h=4),
                        in1=rec[:, :, None].to_broadcast([128, 4, 32]), op=OP.mult)
                    osum = spool.tile([128, C], fp32, tag="osum")
                    nc.vector.tensor_tensor(out=osum[:], in0=scl[:, 0:32],
                                            in1=scl[:, 32:64], op=OP.add)
                    nc.vector.tensor_tensor(out=osum[:], in0=osum[:],
                                            in1=scl[:, 64:96], op=OP.add)
                    nc.vector.tensor_tensor(out=osum[:], in0=osum[:],
                                            in1=scl[:, 96:128], op=OP.add)
                    nc.vector.tensor_tensor(
                        out=osum[:], in0=osum[:],
                        in1=btile[:, layer * C:(layer + 1) * C], op=OP.add)
                    if layer < 2:
                        pst = pstr.tile([32, 128], fp32, tag="pst")
                        nc.tensor.transpose(out=pst[:], in_=osum[:],
                                            identity=ident[:])
                        nc.vector.tensor_copy(
                            out=oT_sb[:, ds(t * 128, 128)], in_=pst[:])
                    else:
                        obf = spool.tile([128, C], bf16, tag="obf")
                        nc.vector.tensor_copy(out=obf[:], in_=osum[:])
                        nc.sync.dma_start(out=out_d[ds(t * 128, 128), :],
                                          in_=obf[:])
    nc.compile()
    return nc


def _inputs_key(inputs):
    h = hashlib.sha1()
    for name in ("edge_index", "W1", "a_src1", "a_dst1", "b1", "W2", "a_src2",
                 "a_dst2", "b2", "W3", "a_src3", "a_dst3", "b3"):
        h.update(np.ascontiguousarray(np.asarray(inputs[name])).tobytes())
    return h.hexdigest()


def kernel(**inputs):
    import ml_dtypes
    from concourse.bass_utils import run_bass_kernel_spmd

    key = _inputs_key(inputs)
    if _CACHE.get("key") != key:
        idx_all, dcol_all, est_all, meta = _host_prep(
            np.asarray(inputs["edge_index"]))
        wdict = _host_weights(inputs)
        nc = _build_program(meta, idx_all, dcol_all, est_all, wdict)
        _CACHE.update(key=key, nc=nc)
    nc = _CACHE["nc"]

    x = np.asarray(inputs["x"], dtype=np.float32)
    in_maps = []
    for k in range(NCORES):
        xk = np.zeros((BLK, 128), dtype=np.float32)
        xk[:NB] = x[k * NB:(k + 1) * NB]
        in_maps.append(
            {"xT": np.ascontiguousarray(xk.T).astype(ml_dtypes.bfloat16)})

    _CACHE["in_maps"] = in_maps
    res = run_bass_kernel_spmd(nc, in_maps, core_ids=list(range(NCORES)))
    out = np.zeros((N, C), dtype=np.float32)
    for k in range(NCORES):
        out[k * NB:(k + 1) * NB] = res.results[k]["out"][:NB].astype(np.float32)
    return out
